# revision 19
# baseline (speedup 1.0000x reference)
"""Dependency-GCN message passing kernel for 8 Trainium2 NeuronCores.

Strategy (destination-sharded, no collectives):
  - Nodes are split into 8 contiguous slices of N/8; core k owns output rows
    [k*N/8, (k+1)*N/8).
  - Every directed message (forward: gov->dep using W[2+r]; reversed:
    dep->gov using W[34+r]) is routed on the host to the core that owns its
    destination node.  x is replicated to every core's HBM.
  - Phase 1 (per core): edges grouped by relation; gather x[src] rows via
    indirect DMA, transpose on PE, matmul with the relation weight, add the
    relation bias via a K=1 ones-outer-product matmul, store messages
    (bf16) to a scratch DRAM buffer laid out by edge slot.
  - Phase 2 (per core): edges sorted by destination; gather message rows in
    dest order, build a one-hot selection matrix A[e, dest] on DVE
    (iota == dstoff), and matmul-accumulate A^T @ M into a PSUM tile per
    128-node destination block.  The per-node self transform x @ W[0] and
    bias b[0] accumulate into the same PSUM tile; ReLU on the way out.
  - Host concatenates the 8 per-core output slices.

All heavy data movement / FLOPs happen on device; the host only shards,
sorts and pads index arrays (and concatenates the final output).
"""

import os

import numpy as np

P = 128  # partitions / tile edge

# bf16 node features / weights on the gather+matmul path (f32 accumulate).
X_BF16 = os.environ.get("GCN_XBF16", "1") == "1"


# ----------------------------------------------------------------------------
# Host-side preparation: shard + sort + pad the edge lists
# ----------------------------------------------------------------------------

def _prepare_host(x, W, b, dep_idx, gov_idx, rel_idx, n_cores):
    import ml_dtypes

    N, D = x.shape
    E = dep_idx.shape[0]
    L = W.shape[0] // 2 - 1          # R = 2 + 2L
    NG = 2 * L                        # directed relation groups
    Npc = N // n_cores

    dep = np.asarray(dep_idx, dtype=np.int64)
    gov = np.asarray(gov_idx, dtype=np.int64)
    rel = np.asarray(rel_idx, dtype=np.int64)

    # Combined directed edge table.  group g in [0, NG): weight = W[2+g].
    grp = np.concatenate([rel - 2, L + (rel - 2)])
    src = np.concatenate([gov, dep])
    dst = np.concatenate([dep, gov])
    core = dst // Npc
    dloc = dst - core * Npc

    # --- pass 1: per-(core, group) and per-(core, block) counts -> paddings
    n_blocks = Npc // P
    gcounts = np.zeros((n_cores, NG), np.int64)
    bcounts = np.zeros((n_cores, n_blocks), np.int64)
    per_core = []
    for k in range(n_cores):
        m = core == k
        gk, sk, dl = grp[m], src[m], dloc[m]
        gcounts[k] = np.bincount(gk, minlength=NG)
        bcounts[k] = np.bincount(dl // P, minlength=n_blocks)
        per_core.append((gk, sk, dl))

    CH_G = int(-(-gcounts.max() // P))          # chunks per relation group
    CH_B = int(-(-bcounts.max() // P))          # chunks per dest block
    CH1 = NG * CH_G                             # phase-1 chunks per core
    CH2 = n_blocks * CH_B                       # phase-2 chunks per core

    shards = []
    for k in range(n_cores):
        gk, sk, dl = per_core[k]
        # ---- phase-1 slot assignment (relation-major) ----
        o1 = np.argsort(gk, kind="stable")
        g1, s1, d1 = gk[o1], sk[o1], dl[o1]
        gstart = np.zeros(NG + 1, np.int64)
        np.cumsum(np.bincount(g1, minlength=NG), out=gstart[1:])
        lpos = np.arange(len(g1)) - gstart[g1]          # pos within group
        c1 = g1 * CH_G + lpos // P
        p1 = lpos % P
        src_packed = np.zeros((P, CH1), np.int32)
        src_packed[p1, c1] = s1
        rowid = (p1 * CH1 + c1).astype(np.int64)         # msgs DRAM row of edge

        # ---- phase-2 slot assignment (destination-major) ----
        o2 = np.argsort(d1, kind="stable")
        d2, r2 = d1[o2], rowid[o2]
        blk = d2 // P
        bstart = np.zeros(n_blocks + 1, np.int64)
        np.cumsum(np.bincount(blk, minlength=n_blocks), out=bstart[1:])
        bpos = np.arange(len(d2)) - bstart[blk]
        c2 = blk * CH_B + bpos // P
        p2 = bpos % P
        perm_packed = np.zeros((P, CH2), np.int32)
        dstoff_packed = np.full((P, CH2), 255, np.float32)
        perm_packed[p2, c2] = r2
        dstoff_packed[p2, c2] = d2 % P

        shards.append({
            "src": src_packed,
            "perm": perm_packed,
            "dstoff": dstoff_packed,
            "xself": np.ascontiguousarray(x[k * Npc:(k + 1) * Npc]),
        })

    xdt = ml_dtypes.bfloat16 if X_BF16 else np.float32
    wstk = np.ascontiguousarray(
        np.transpose(W[2:2 + NG], (1, 0, 2)).reshape(D, NG * D).astype(xdt))
    w0 = np.ascontiguousarray(W[0].astype(xdt))
    bstk = np.ascontiguousarray(
        b[2:2 + NG].reshape(1, NG * D).astype(ml_dtypes.bfloat16))
    b0 = np.ascontiguousarray(b[0].reshape(1, D).astype(ml_dtypes.bfloat16))

    shared = {"x": np.ascontiguousarray(x.astype(xdt)),
              "wstk": wstk, "w0": w0, "bstk": bstk, "b0": b0}
    for sh in shards:
        sh["xself"] = sh["xself"].astype(xdt)
    meta = dict(N=N, D=D, NG=NG, Npc=Npc, n_blocks=n_blocks,
                CH_G=CH_G, CH_B=CH_B, CH1=CH1, CH2=CH2)
    return shared, shards, meta


# ----------------------------------------------------------------------------
# Device program
# ----------------------------------------------------------------------------

def _build_program(meta, bench_reps=0):
    import contextlib
    import concourse.bass as bass
    import concourse.bacc as bacc
    import concourse.mybir as mybir
    import concourse.tile as tile
    from concourse.masks import make_identity

    D = meta["D"]
    NG = meta["NG"]
    N = meta["N"]
    Npc = meta["Npc"]
    n_blocks = meta["n_blocks"]
    CH_G, CH_B, CH1, CH2 = meta["CH_G"], meta["CH_B"], meta["CH1"], meta["CH2"]

    f32 = mybir.dt.float32
    bf16 = mybir.dt.bfloat16
    i32 = mybir.dt.int32
    xdt = bf16 if X_BF16 else f32

    # phase-1 pieces: chunks per gather piece
    PC1 = 32
    while CH1 % PC1:
        PC1 //= 2
    NP1 = CH1 // PC1
    # phase-2 pieces: whole number of blocks per piece
    BPP = 8
    while n_blocks % BPP:
        BPP //= 2
    PC2 = BPP * CH_B
    NP2 = n_blocks // BPP

    nc = bacc.Bacc(None, target_bir_lowering=False, num_swdge_queues=4)

    x_d = nc.dram_tensor("x", [N, D], xdt, kind="ExternalInput")
    xself_d = nc.dram_tensor("xself", [Npc, D], xdt, kind="ExternalInput")
    wstk_d = nc.dram_tensor("wstk", [D, NG * D], xdt, kind="ExternalInput")
    w0_d = nc.dram_tensor("w0", [D, D], xdt, kind="ExternalInput")
    bstk_d = nc.dram_tensor("bstk", [1, NG * D], bf16, kind="ExternalInput")
    b0_d = nc.dram_tensor("b0", [1, D], bf16, kind="ExternalInput")
    src_d = nc.dram_tensor("src", [P, CH1], i32, kind="ExternalInput")
    perm_d = nc.dram_tensor("perm", [P, CH2], i32, kind="ExternalInput")
    dstoff_d = nc.dram_tensor("dstoff", [P, CH2], f32, kind="ExternalInput")
    msgs_d = nc.dram_tensor("msgs", [P * CH1, D], bf16, kind="Internal")
    out_d = nc.dram_tensor("out", [Npc, D], f32, kind="ExternalOutput")

    msgs_v = msgs_d[:, :].rearrange("(p c) d -> p c d", p=P)
    xself_v = xself_d[:, :].rearrange("(b p) d -> p b d", p=P)
    out_v = out_d[:, :].rearrange("(b p) d -> p b d", p=P)

    with tile.TileContext(nc) as tc:
        with tc.tile_pool(name="const", bufs=1) as cpool:
            ident = cpool.tile([P, P], xdt)
            make_identity(nc, ident[:])
            iota_t = cpool.tile([P, P], f32)
            nc.gpsimd.iota(iota_t[:], pattern=[[1, P]], base=0,
                           channel_multiplier=0,
                           allow_small_or_imprecise_dtypes=True)
            ones_t = cpool.tile([1, P], bf16)
            nc.vector.memset(ones_t[:], 1.0)
            wsb = cpool.tile([P, NG * D], xdt)
            nc.sync.dma_start(out=wsb[:], in_=wstk_d[:, :])
            w0sb = cpool.tile([P, D], xdt)
            nc.sync.dma_start(out=w0sb[:], in_=w0_d[:, :])
            bsb = cpool.tile([1, NG * D], bf16)
            nc.sync.dma_start(out=bsb[:], in_=bstk_d[:, :])
            b0sb = cpool.tile([1, D], bf16)
            nc.sync.dma_start(out=b0sb[:], in_=b0_d[:, :])
            srcsb = cpool.tile([P, CH1], i32)
            nc.sync.dma_start(out=srcsb[:], in_=src_d[:, :])
            permsb = cpool.tile([P, CH2], i32)
            nc.sync.dma_start(out=permsb[:], in_=perm_d[:, :])
            dstsb = cpool.tile([P, CH2], f32)
            nc.sync.dma_start(out=dstsb[:], in_=dstoff_d[:, :])

            loop_cm = (tc.For_i(0, bench_reps, 1) if bench_reps
                       else contextlib.nullcontext())
            with loop_cm:
                _build_body(nc, tc, meta, locals())

    nc.finalize()
    return nc


def _build_body(nc, tc, meta, env):
    import concourse.bass as bass
    import concourse.mybir as mybir

    D = meta["D"]
    CH_G, CH_B = meta["CH_G"], meta["CH_B"]
    PC1, NP1 = env["PC1"], env["NP1"]
    PC2, NP2, BPP = env["PC2"], env["NP2"], env["BPP"]
    f32, bf16 = mybir.dt.float32, mybir.dt.bfloat16
    xdt = env["xdt"]
    x_d, msgs_d = env["x_d"], env["msgs_d"]
    msgs_v, xself_v, out_v = env["msgs_v"], env["xself_v"], env["out_v"]
    ident, iota_t, ones_t = env["ident"], env["iota_t"], env["ones_t"]
    wsb, w0sb, bsb, b0sb = env["wsb"], env["w0sb"], env["bsb"], env["b0sb"]
    srcsb, permsb, dstsb = env["srcsb"], env["permsb"], env["dstsb"]

    if True:
        if True:
            # ---------------- phase 1: messages ----------------
            with tc.tile_pool(name="p1big", bufs=2) as p1big, \
                 tc.tile_pool(name="p1small", bufs=10) as p1small, \
                 tc.tile_pool(name="ps_t", bufs=3, space="PSUM") as ps_t, \
                 tc.tile_pool(name="ps_m", bufs=3, space="PSUM") as ps_m:
                for piece in range(NP1):
                    mp = p1big.tile([P, PC1 * D], bf16, tag="mp")
                    for cl in range(PC1):
                        c = piece * PC1 + cl
                        g = c // CH_G
                        xg = p1small.tile([P, D], xdt, tag="xg")
                        _gi = nc.gpsimd.indirect_dma_start(
                            out=xg[:],
                            out_offset=None,
                            in_=x_d[:, :],
                            in_offset=bass.IndirectOffsetOnAxis(
                                ap=srcsb[:, c:c + 1], axis=0),
                        )
                        _q = c % 4
                        _gi.ins.queue = f"qPoolDynamic{_q if _q else ''}"
                        xgT_ps = ps_t.tile([P, P], xdt, tag="t")
                        nc.tensor.transpose(xgT_ps[:], xg[:], ident[:])
                        xgT = p1small.tile([P, P], xdt, tag="xgT")
                        nc.any.tensor_copy(xgT[:], xgT_ps[:])
                        mm_ps = ps_m.tile([P, P], f32, tag="m")
                        nc.tensor.matmul(
                            mm_ps[:], lhsT=xgT[:],
                            rhs=wsb[:, g * D:(g + 1) * D],
                            start=True, stop=False)
                        nc.tensor.matmul(
                            mm_ps[:], lhsT=ones_t[:],
                            rhs=bsb[:, g * D:(g + 1) * D],
                            start=False, stop=True)
                        nc.any.tensor_copy(mp[:, cl * D:(cl + 1) * D], mm_ps[:])
                    nc.sync.dma_start(
                        out=msgs_v[:, piece * PC1:(piece + 1) * PC1, :],
                        in_=mp[:])

            # ---------------- phase 2: aggregate ----------------
            with tc.tile_pool(name="p2big", bufs=2) as p2big, \
                 tc.tile_pool(name="p2small", bufs=10) as p2small, \
                 tc.tile_pool(name="ps_t2", bufs=2, space="PSUM") as ps_t2, \
                 tc.tile_pool(name="ps_o", bufs=3, space="PSUM") as ps_o:
                for piece in range(NP2):
                    xs = p2big.tile([P, BPP * D], xdt, tag="xs")
                    nc.sync.dma_start(
                        out=xs[:],
                        in_=xself_v[:, piece * BPP:(piece + 1) * BPP, :])
                    ob = p2big.tile([P, BPP * D], f32, tag="ob")
                    for bl in range(BPP):
                        blk_ps = ps_o.tile([P, P], f32, tag="o")
                        # self transform
                        xsT_ps = ps_t2.tile([P, P], xdt, tag="t2")
                        nc.tensor.transpose(
                            xsT_ps[:], xs[:, bl * D:(bl + 1) * D], ident[:])
                        xsT = p2small.tile([P, P], xdt, tag="xsT")
                        nc.any.tensor_copy(xsT[:], xsT_ps[:])
                        nc.tensor.matmul(blk_ps[:], lhsT=xsT[:], rhs=w0sb[:],
                                         start=True, stop=False)
                        nc.tensor.matmul(blk_ps[:], lhsT=ones_t[:],
                                         rhs=b0sb[:], start=False, stop=False)
                        for j in range(CH_B):
                            c2 = piece * PC2 + bl * CH_B + j
                            mg = p2small.tile([P, D], bf16, tag="mg")
                            _gi2 = nc.gpsimd.indirect_dma_start(
                                out=mg[:],
                                out_offset=None,
                                in_=msgs_d[:, :],
                                in_offset=bass.IndirectOffsetOnAxis(
                                    ap=permsb[:, c2:c2 + 1], axis=0),
                            )
                            _q2 = c2 % 4
                            _gi2.ins.queue = f"qPoolDynamic{_q2 if _q2 else ''}"
                            a_t = p2small.tile([P, P], bf16, tag="a")
                            nc.vector.tensor_scalar(
                                out=a_t[:], in0=iota_t[:],
                                scalar1=dstsb[:, c2:c2 + 1],
                                scalar2=None,
                                op0=mybir.AluOpType.is_equal)
                            nc.tensor.matmul(
                                blk_ps[:], lhsT=a_t[:],
                                rhs=mg[:],
                                start=False, stop=(j == CH_B - 1))
                        nc.scalar.activation(
                            ob[:, bl * D:(bl + 1) * D], blk_ps[:],
                            mybir.ActivationFunctionType.Relu)
                    nc.sync.dma_start(
                        out=out_v[:, piece * BPP:(piece + 1) * BPP, :],
                        in_=ob[:])


# ----------------------------------------------------------------------------
# Entry point
# ----------------------------------------------------------------------------

def _run_timed(nc, in_maps, n_cores=8, iters=1):
    """Execute a finalized Bass program via PJRT on n_cores devices.

    Returns (per-core results, best wall seconds of a repeat execution).
    """
    import time
    import jax
    from jax.sharding import Mesh, PartitionSpec, NamedSharding
    from jax.experimental.shard_map import shard_map
    import concourse.mybir as mybir
    from concourse import bass2jax

    bass2jax.install_neuronx_cc_hook()
    partition_name = (nc.partition_id_tensor.name
                      if nc.partition_id_tensor else None)

    in_names, out_names, out_avals, zero_outs = [], [], [], []
    for alloc in nc.m.functions[0].allocations:
        if not isinstance(alloc, mybir.MemoryLocationSet):
            continue
        name = alloc.memorylocations[0].name
        if alloc.kind == "ExternalInput":
            if name != partition_name:
                in_names.append(name)
        elif alloc.kind == "ExternalOutput":
            out_names.append(name)
            shape = tuple(alloc.tensor_shape)
            dtype = mybir.dt.np(alloc.dtype)
            out_avals.append(jax.core.ShapedArray(shape, dtype))
            zero_outs.append(np.zeros(shape, dtype))
    n_params = len(in_names)
    n_outs = len(out_avals)
    all_names = in_names + out_names
    if partition_name is not None:
        all_names = all_names + [partition_name]

    def _body(*args):
        operands = list(args)
        if partition_name is not None:
            operands.append(bass2jax.partition_id_tensor())
        outs = bass2jax._bass_exec_p.bind(
            *operands,
            out_avals=tuple(out_avals),
            in_names=tuple(all_names),
            out_names=tuple(out_names),
            lowering_input_output_aliases=(),
            sim_require_finite=True,
            sim_require_nnan=True,
            nc=nc,
        )
        return tuple(outs)

    devices = jax.devices()[:n_cores]
    mesh = Mesh(np.asarray(devices), ("core",))
    spec = PartitionSpec("core")
    sharded = jax.jit(
        shard_map(_body, mesh=mesh, in_specs=(spec,) * (n_params + n_outs),
                  out_specs=(spec,) * n_outs, check_rep=False),
        keep_unused=True)

    sh = NamedSharding(mesh, spec)
    concat_in = [
        jax.device_put(
            np.concatenate([np.asarray(in_maps[c][n]) for c in range(n_cores)],
                           axis=0), sh)
        for n in in_names
    ]
    concat_zeros = [
        jax.device_put(np.zeros((n_cores * z.shape[0], *z.shape[1:]), z.dtype),
                       sh)
        for z in zero_outs
    ]

    out_arrs = jax.block_until_ready(sharded(*concat_in, *concat_zeros))
    best = float("inf")
    for _ in range(iters):
        t0 = time.perf_counter()
        out_arrs = jax.block_until_ready(sharded(*concat_in, *concat_zeros))
        best = min(best, time.perf_counter() - t0)

    results = [
        {name: np.asarray(out_arrs[i]).reshape(n_cores, *out_avals[i].shape)[c]
         for i, name in enumerate(out_names)}
        for c in range(n_cores)
    ]
    return results, best


last_exec_seconds = None  # wall seconds of the most recent run (for test.py)


def kernel(x, W, b, dep_idx, gov_idx, rel_idx):
    n_cores = 8
    shared, shards, meta = _prepare_host(
        np.asarray(x), np.asarray(W), np.asarray(b),
        dep_idx, gov_idx, rel_idx, n_cores)

    nc = _build_program(meta)

    in_maps = [dict(shared, **sh) for sh in shards]
    results, best = _run_timed(nc, in_maps, n_cores=n_cores, iters=3)
    global last_exec_seconds
    last_exec_seconds = best
    out = np.concatenate([r["out"] for r in results], axis=0)
    return out.astype(np.float32)


# revision 20
# speedup vs baseline: 1.3218x; 1.3218x over previous
"""Dependency-GCN message passing kernel for 8 Trainium2 NeuronCores.

Strategy (destination-sharded, no collectives):
  - Nodes are split into 8 contiguous slices of N/8; core k owns output rows
    [k*N/8, (k+1)*N/8).
  - Every directed message (forward: gov->dep using W[2+r]; reversed:
    dep->gov using W[34+r]) is routed on the host to the core that owns its
    destination node.  x is replicated to every core's HBM.
  - Phase 1 (per core): edges grouped by relation; gather x[src] rows via
    indirect DMA, transpose on PE, matmul with the relation weight, add the
    relation bias via a K=1 ones-outer-product matmul, store messages
    (bf16) to a scratch DRAM buffer laid out by edge slot.
  - Phase 2 (per core): edges sorted by destination; gather message rows in
    dest order, build a one-hot selection matrix A[e, dest] on DVE
    (iota == dstoff), and matmul-accumulate A^T @ M into a PSUM tile per
    128-node destination block.  The per-node self transform x @ W[0] and
    bias b[0] accumulate into the same PSUM tile; ReLU on the way out.
  - Host concatenates the 8 per-core output slices.

All heavy data movement / FLOPs happen on device; the host only shards,
sorts and pads index arrays (and concatenates the final output).
"""

import os

import numpy as np

P = 128  # partitions / tile edge

# bf16 node features / weights on the gather+matmul path (f32 accumulate).
X_BF16 = os.environ.get("GCN_XBF16", "1") == "1"


# ----------------------------------------------------------------------------
# Host-side preparation: shard + sort + pad the edge lists
# ----------------------------------------------------------------------------

def _prepare_host(x, W, b, dep_idx, gov_idx, rel_idx, n_cores):
    import ml_dtypes

    N, D = x.shape
    E = dep_idx.shape[0]
    L = W.shape[0] // 2 - 1          # R = 2 + 2L
    NG = 2 * L                        # directed relation groups
    Npc = N // n_cores

    dep = np.asarray(dep_idx, dtype=np.int64)
    gov = np.asarray(gov_idx, dtype=np.int64)
    rel = np.asarray(rel_idx, dtype=np.int64)

    # Combined directed edge table.  group g in [0, NG): weight = W[2+g].
    grp = np.concatenate([rel - 2, L + (rel - 2)])
    src = np.concatenate([gov, dep])
    dst = np.concatenate([dep, gov])
    core = dst // Npc
    dloc = dst - core * Npc

    # --- pass 1: per-(core, group) and per-(core, block) counts -> paddings
    n_blocks = Npc // P
    gcounts = np.zeros((n_cores, NG), np.int64)
    bcounts = np.zeros((n_cores, n_blocks), np.int64)
    per_core = []
    for k in range(n_cores):
        m = core == k
        gk, sk, dl = grp[m], src[m], dloc[m]
        gcounts[k] = np.bincount(gk, minlength=NG)
        bcounts[k] = np.bincount(dl // P, minlength=n_blocks)
        per_core.append((gk, sk, dl))

    CH_G = int(-(-gcounts.max() // P))          # chunks per relation group
    CH_B = int(-(-bcounts.max() // P))          # chunks per dest block
    CH1 = NG * CH_G                             # phase-1 chunks per core
    CH2 = n_blocks * CH_B                       # phase-2 chunks per core

    shards = []
    for k in range(n_cores):
        gk, sk, dl = per_core[k]
        # ---- phase-1 slot assignment (relation-major) ----
        o1 = np.argsort(gk, kind="stable")
        g1, s1, d1 = gk[o1], sk[o1], dl[o1]
        gstart = np.zeros(NG + 1, np.int64)
        np.cumsum(np.bincount(g1, minlength=NG), out=gstart[1:])
        lpos = np.arange(len(g1)) - gstart[g1]          # pos within group
        c1 = g1 * CH_G + lpos // P
        p1 = lpos % P
        src_packed = np.zeros((P, CH1), np.int32)
        src_packed[p1, c1] = s1
        rowid = (p1 * CH1 + c1).astype(np.int64)         # msgs DRAM row of edge

        # ---- phase-2 slot assignment (destination-major) ----
        o2 = np.argsort(d1, kind="stable")
        d2, r2 = d1[o2], rowid[o2]
        blk = d2 // P
        bstart = np.zeros(n_blocks + 1, np.int64)
        np.cumsum(np.bincount(blk, minlength=n_blocks), out=bstart[1:])
        bpos = np.arange(len(d2)) - bstart[blk]
        c2 = blk * CH_B + bpos // P
        p2 = bpos % P
        perm_packed = np.zeros((P, CH2), np.int32)
        dstoff_packed = np.full((P, CH2), 255, np.float32)
        perm_packed[p2, c2] = r2
        dstoff_packed[p2, c2] = d2 % P

        shards.append({
            "src": src_packed,
            "perm": perm_packed,
            "dstoff": dstoff_packed,
            "xself": np.ascontiguousarray(x[k * Npc:(k + 1) * Npc]),
        })

    xdt = ml_dtypes.bfloat16 if X_BF16 else np.float32
    wstk = np.ascontiguousarray(
        np.transpose(W[2:2 + NG], (1, 0, 2)).reshape(D, NG * D).astype(xdt))
    w0 = np.ascontiguousarray(W[0].astype(xdt))
    bstk = np.ascontiguousarray(
        b[2:2 + NG].reshape(1, NG * D).astype(ml_dtypes.bfloat16))
    b0 = np.ascontiguousarray(b[0].reshape(1, D).astype(ml_dtypes.bfloat16))

    shared = {"x": np.ascontiguousarray(x.astype(xdt)),
              "wstk": wstk, "w0": w0, "bstk": bstk, "b0": b0}
    for sh in shards:
        sh["xself"] = sh["xself"].astype(xdt)
    meta = dict(N=N, D=D, NG=NG, Npc=Npc, n_blocks=n_blocks,
                CH_G=CH_G, CH_B=CH_B, CH1=CH1, CH2=CH2)
    return shared, shards, meta


# ----------------------------------------------------------------------------
# Device program
# ----------------------------------------------------------------------------

def _build_program(meta, bench_reps=0):
    import contextlib
    import concourse.bass as bass
    import concourse.bacc as bacc
    import concourse.mybir as mybir
    import concourse.tile as tile
    from concourse.masks import make_identity

    D = meta["D"]
    NG = meta["NG"]
    N = meta["N"]
    Npc = meta["Npc"]
    n_blocks = meta["n_blocks"]
    CH_G, CH_B, CH1, CH2 = meta["CH_G"], meta["CH_B"], meta["CH1"], meta["CH2"]

    f32 = mybir.dt.float32
    bf16 = mybir.dt.bfloat16
    i32 = mybir.dt.int32
    xdt = bf16 if X_BF16 else f32

    # phase-1 pieces: chunks per gather piece
    PC1 = 32
    while CH1 % PC1:
        PC1 //= 2
    NP1 = CH1 // PC1
    # phase-2 pieces: whole number of blocks per piece
    BPP = 8
    while n_blocks % BPP:
        BPP //= 2
    PC2 = BPP * CH_B
    NP2 = n_blocks // BPP

    nc = bacc.Bacc(None, target_bir_lowering=False)

    x_d = nc.dram_tensor("x", [N, D], xdt, kind="ExternalInput")
    xself_d = nc.dram_tensor("xself", [Npc, D], xdt, kind="ExternalInput")
    wstk_d = nc.dram_tensor("wstk", [D, NG * D], xdt, kind="ExternalInput")
    w0_d = nc.dram_tensor("w0", [D, D], xdt, kind="ExternalInput")
    bstk_d = nc.dram_tensor("bstk", [1, NG * D], bf16, kind="ExternalInput")
    b0_d = nc.dram_tensor("b0", [1, D], bf16, kind="ExternalInput")
    src_d = nc.dram_tensor("src", [P, CH1], i32, kind="ExternalInput")
    perm_d = nc.dram_tensor("perm", [P, CH2], i32, kind="ExternalInput")
    dstoff_d = nc.dram_tensor("dstoff", [P, CH2], f32, kind="ExternalInput")
    msgs_d = nc.dram_tensor("msgs", [P * CH1, D], bf16, kind="Internal")
    out_d = nc.dram_tensor("out", [Npc, D], f32, kind="ExternalOutput")

    msgs_v = msgs_d[:, :].rearrange("(p c) d -> p c d", p=P)
    xself_v = xself_d[:, :].rearrange("(b p) d -> p b d", p=P)
    out_v = out_d[:, :].rearrange("(b p) d -> p b d", p=P)

    with tile.TileContext(nc) as tc:
        with tc.tile_pool(name="const", bufs=1) as cpool:
            ident = cpool.tile([P, P], xdt)
            make_identity(nc, ident[:])
            iota_t = cpool.tile([P, P], f32)
            nc.gpsimd.iota(iota_t[:], pattern=[[1, P]], base=0,
                           channel_multiplier=0,
                           allow_small_or_imprecise_dtypes=True)
            ones_t = cpool.tile([1, P], bf16)
            nc.vector.memset(ones_t[:], 1.0)
            wsb = cpool.tile([P, NG * D], xdt)
            nc.sync.dma_start(out=wsb[:], in_=wstk_d[:, :])
            w0sb = cpool.tile([P, D], xdt)
            nc.sync.dma_start(out=w0sb[:], in_=w0_d[:, :])
            bsb = cpool.tile([1, NG * D], bf16)
            nc.sync.dma_start(out=bsb[:], in_=bstk_d[:, :])
            b0sb = cpool.tile([1, D], bf16)
            nc.sync.dma_start(out=b0sb[:], in_=b0_d[:, :])
            srcsb = cpool.tile([P, CH1], i32)
            nc.sync.dma_start(out=srcsb[:], in_=src_d[:, :])
            permsb = cpool.tile([P, CH2], i32)
            nc.sync.dma_start(out=permsb[:], in_=perm_d[:, :])
            dstsb = cpool.tile([P, CH2], f32)
            nc.sync.dma_start(out=dstsb[:], in_=dstoff_d[:, :])

            loop_cm = (tc.For_i(0, bench_reps, 1) if bench_reps
                       else contextlib.nullcontext())
            with loop_cm:
                _build_body(nc, tc, meta, locals())

    nc.finalize()
    return nc


def _build_body(nc, tc, meta, env):
    import concourse.bass as bass
    import concourse.mybir as mybir

    D = meta["D"]
    CH_G, CH_B = meta["CH_G"], meta["CH_B"]
    PC1, NP1 = env["PC1"], env["NP1"]
    PC2, NP2, BPP = env["PC2"], env["NP2"], env["BPP"]
    f32, bf16 = mybir.dt.float32, mybir.dt.bfloat16
    xdt = env["xdt"]
    x_d, msgs_d = env["x_d"], env["msgs_d"]
    msgs_v, xself_v, out_v = env["msgs_v"], env["xself_v"], env["out_v"]
    ident, iota_t, ones_t = env["ident"], env["iota_t"], env["ones_t"]
    wsb, w0sb, bsb, b0sb = env["wsb"], env["w0sb"], env["bsb"], env["b0sb"]
    srcsb, permsb, dstsb = env["srcsb"], env["permsb"], env["dstsb"]

    if True:
        if True:
            # ---------------- phase 1: messages ----------------
            with tc.tile_pool(name="p1big", bufs=2) as p1big, \
                 tc.tile_pool(name="p1small", bufs=10) as p1small, \
                 tc.tile_pool(name="ps_t", bufs=3, space="PSUM") as ps_t, \
                 tc.tile_pool(name="ps_m", bufs=3, space="PSUM") as ps_m:
                for piece in range(NP1):
                    mp = p1big.tile([P, PC1 * D], bf16, tag="mp")
                    for cl in range(PC1):
                        c = piece * PC1 + cl
                        g = c // CH_G
                        xg = p1small.tile([P, D], xdt, tag="xg")
                        nc.gpsimd.indirect_dma_start(
                            out=xg[:],
                            out_offset=None,
                            in_=x_d[:, :],
                            in_offset=bass.IndirectOffsetOnAxis(
                                ap=srcsb[:, c:c + 1], axis=0),
                        )
                        xgT_ps = ps_t.tile([P, P], xdt, tag="t")
                        nc.tensor.transpose(xgT_ps[:], xg[:], ident[:])
                        xgT = p1small.tile([P, P], xdt, tag="xgT")
                        nc.any.tensor_copy(xgT[:], xgT_ps[:])
                        mm_ps = ps_m.tile([P, P], f32, tag="m")
                        nc.tensor.matmul(
                            mm_ps[:], lhsT=xgT[:],
                            rhs=wsb[:, g * D:(g + 1) * D],
                            start=True, stop=False)
                        nc.tensor.matmul(
                            mm_ps[:], lhsT=ones_t[:],
                            rhs=bsb[:, g * D:(g + 1) * D],
                            start=False, stop=True)
                        nc.any.tensor_copy(mp[:, cl * D:(cl + 1) * D], mm_ps[:])
                    nc.sync.dma_start(
                        out=msgs_v[:, piece * PC1:(piece + 1) * PC1, :],
                        in_=mp[:])

            # ---------------- phase 2: aggregate ----------------
            with tc.tile_pool(name="p2big", bufs=2) as p2big, \
                 tc.tile_pool(name="p2small", bufs=10) as p2small, \
                 tc.tile_pool(name="ps_t2", bufs=2, space="PSUM") as ps_t2, \
                 tc.tile_pool(name="ps_o", bufs=3, space="PSUM") as ps_o:
                for piece in range(NP2):
                    xs = p2big.tile([P, BPP * D], xdt, tag="xs")
                    nc.sync.dma_start(
                        out=xs[:],
                        in_=xself_v[:, piece * BPP:(piece + 1) * BPP, :])
                    ob = p2big.tile([P, BPP * D], f32, tag="ob")
                    for bl in range(BPP):
                        blk_ps = ps_o.tile([P, P], f32, tag="o")
                        # self transform
                        xsT_ps = ps_t2.tile([P, P], xdt, tag="t2")
                        nc.tensor.transpose(
                            xsT_ps[:], xs[:, bl * D:(bl + 1) * D], ident[:])
                        xsT = p2small.tile([P, P], xdt, tag="xsT")
                        nc.any.tensor_copy(xsT[:], xsT_ps[:])
                        nc.tensor.matmul(blk_ps[:], lhsT=xsT[:], rhs=w0sb[:],
                                         start=True, stop=False)
                        nc.tensor.matmul(blk_ps[:], lhsT=ones_t[:],
                                         rhs=b0sb[:], start=False, stop=False)
                        for j in range(CH_B):
                            c2 = piece * PC2 + bl * CH_B + j
                            mg = p2small.tile([P, D], bf16, tag="mg")
                            nc.gpsimd.indirect_dma_start(
                                out=mg[:],
                                out_offset=None,
                                in_=msgs_d[:, :],
                                in_offset=bass.IndirectOffsetOnAxis(
                                    ap=permsb[:, c2:c2 + 1], axis=0),
                            )
                            a_t = p2small.tile([P, P], bf16, tag="a")
                            nc.vector.tensor_scalar(
                                out=a_t[:], in0=iota_t[:],
                                scalar1=dstsb[:, c2:c2 + 1],
                                scalar2=None,
                                op0=mybir.AluOpType.is_equal)
                            nc.tensor.matmul(
                                blk_ps[:], lhsT=a_t[:],
                                rhs=mg[:],
                                start=False, stop=(j == CH_B - 1))
                        nc.scalar.activation(
                            ob[:, bl * D:(bl + 1) * D], blk_ps[:],
                            mybir.ActivationFunctionType.Relu)
                    nc.sync.dma_start(
                        out=out_v[:, piece * BPP:(piece + 1) * BPP, :],
                        in_=ob[:])


# ----------------------------------------------------------------------------
# Entry point
# ----------------------------------------------------------------------------

def _run_timed(nc, in_maps, n_cores=8, iters=1):
    """Execute a finalized Bass program via PJRT on n_cores devices.

    Returns (per-core results, best wall seconds of a repeat execution).
    """
    import time
    import jax
    from jax.sharding import Mesh, PartitionSpec, NamedSharding
    from jax.experimental.shard_map import shard_map
    import concourse.mybir as mybir
    from concourse import bass2jax

    bass2jax.install_neuronx_cc_hook()
    partition_name = (nc.partition_id_tensor.name
                      if nc.partition_id_tensor else None)

    in_names, out_names, out_avals, zero_outs = [], [], [], []
    for alloc in nc.m.functions[0].allocations:
        if not isinstance(alloc, mybir.MemoryLocationSet):
            continue
        name = alloc.memorylocations[0].name
        if alloc.kind == "ExternalInput":
            if name != partition_name:
                in_names.append(name)
        elif alloc.kind == "ExternalOutput":
            out_names.append(name)
            shape = tuple(alloc.tensor_shape)
            dtype = mybir.dt.np(alloc.dtype)
            out_avals.append(jax.core.ShapedArray(shape, dtype))
            zero_outs.append(np.zeros(shape, dtype))
    n_params = len(in_names)
    n_outs = len(out_avals)
    all_names = in_names + out_names
    if partition_name is not None:
        all_names = all_names + [partition_name]

    def _body(*args):
        operands = list(args)
        if partition_name is not None:
            operands.append(bass2jax.partition_id_tensor())
        outs = bass2jax._bass_exec_p.bind(
            *operands,
            out_avals=tuple(out_avals),
            in_names=tuple(all_names),
            out_names=tuple(out_names),
            lowering_input_output_aliases=(),
            sim_require_finite=True,
            sim_require_nnan=True,
            nc=nc,
        )
        return tuple(outs)

    devices = jax.devices()[:n_cores]
    mesh = Mesh(np.asarray(devices), ("core",))
    spec = PartitionSpec("core")
    sharded = jax.jit(
        shard_map(_body, mesh=mesh, in_specs=(spec,) * (n_params + n_outs),
                  out_specs=(spec,) * n_outs, check_rep=False),
        keep_unused=True)

    sh = NamedSharding(mesh, spec)
    concat_in = [
        jax.device_put(
            np.concatenate([np.asarray(in_maps[c][n]) for c in range(n_cores)],
                           axis=0), sh)
        for n in in_names
    ]
    concat_zeros = [
        jax.device_put(np.zeros((n_cores * z.shape[0], *z.shape[1:]), z.dtype),
                       sh)
        for z in zero_outs
    ]

    out_arrs = jax.block_until_ready(sharded(*concat_in, *concat_zeros))
    best = float("inf")
    for _ in range(iters):
        t0 = time.perf_counter()
        out_arrs = jax.block_until_ready(sharded(*concat_in, *concat_zeros))
        best = min(best, time.perf_counter() - t0)

    results = [
        {name: np.asarray(out_arrs[i]).reshape(n_cores, *out_avals[i].shape)[c]
         for i, name in enumerate(out_names)}
        for c in range(n_cores)
    ]
    return results, best


last_exec_seconds = None  # wall seconds of the most recent run (for test.py)


def kernel(x, W, b, dep_idx, gov_idx, rel_idx):
    n_cores = 8
    shared, shards, meta = _prepare_host(
        np.asarray(x), np.asarray(W), np.asarray(b),
        dep_idx, gov_idx, rel_idx, n_cores)

    nc = _build_program(meta)

    in_maps = [dict(shared, **sh) for sh in shards]
    results, best = _run_timed(nc, in_maps, n_cores=n_cores, iters=3)
    global last_exec_seconds
    last_exec_seconds = best
    out = np.concatenate([r["out"] for r in results], axis=0)
    return out.astype(np.float32)


# revision 21
# speedup vs baseline: 1.3401x; 1.0139x over previous
"""Dependency-GCN message passing kernel for 8 Trainium2 NeuronCores.

Strategy (destination-sharded, no collectives):
  - Nodes are split into 8 contiguous slices of N/8; core k owns output rows
    [k*N/8, (k+1)*N/8).
  - Every directed message (forward: gov->dep using W[2+r]; reversed:
    dep->gov using W[34+r]) is routed on the host to the core that owns its
    destination node.  x is replicated to every core's HBM.
  - Phase 1 (per core): edges grouped by relation; gather x[src] rows via
    indirect DMA, transpose on PE, matmul with the relation weight, add the
    relation bias via a K=1 ones-outer-product matmul, store messages
    (bf16) to a scratch DRAM buffer laid out by edge slot.
  - Phase 2 (per core): edges sorted by destination; gather message rows in
    dest order, build a one-hot selection matrix A[e, dest] on DVE
    (iota == dstoff), and matmul-accumulate A^T @ M into a PSUM tile per
    128-node destination block.  The per-node self transform x @ W[0] and
    bias b[0] accumulate into the same PSUM tile; ReLU on the way out.
  - Host concatenates the 8 per-core output slices.

All heavy data movement / FLOPs happen on device; the host only shards,
sorts and pads index arrays (and concatenates the final output).
"""

import os

import numpy as np

P = 128  # partitions / tile edge

# bf16 node features / weights on the gather+matmul path (f32 accumulate).
X_BF16 = os.environ.get("GCN_XBF16", "1") == "1"


# ----------------------------------------------------------------------------
# Host-side preparation: shard + sort + pad the edge lists
# ----------------------------------------------------------------------------

def _prepare_host(x, W, b, dep_idx, gov_idx, rel_idx, n_cores):
    import ml_dtypes

    N, D = x.shape
    E = dep_idx.shape[0]
    L = W.shape[0] // 2 - 1          # R = 2 + 2L
    NG = 2 * L                        # directed relation groups
    Npc = N // n_cores

    dep = np.asarray(dep_idx, dtype=np.int64)
    gov = np.asarray(gov_idx, dtype=np.int64)
    rel = np.asarray(rel_idx, dtype=np.int64)

    # Combined directed edge table.  group g in [0, NG): weight = W[2+g].
    grp = np.concatenate([rel - 2, L + (rel - 2)])
    src = np.concatenate([gov, dep])
    dst = np.concatenate([dep, gov])
    core = dst // Npc
    dloc = dst - core * Npc

    # --- pass 1: per-(core, group) and per-(core, block) counts -> paddings
    n_blocks = Npc // P
    gcounts = np.zeros((n_cores, NG), np.int64)
    bcounts = np.zeros((n_cores, n_blocks), np.int64)
    per_core = []
    for k in range(n_cores):
        m = core == k
        gk, sk, dl = grp[m], src[m], dloc[m]
        gcounts[k] = np.bincount(gk, minlength=NG)
        bcounts[k] = np.bincount(dl // P, minlength=n_blocks)
        per_core.append((gk, sk, dl))

    CH_G = int(-(-gcounts.max() // P))          # chunks per relation group
    CH_B = int(-(-bcounts.max() // P))          # chunks per dest block
    CH1 = NG * CH_G                             # phase-1 chunks per core
    CH2 = n_blocks * CH_B                       # phase-2 chunks per core

    shards = []
    for k in range(n_cores):
        gk, sk, dl = per_core[k]
        # ---- phase-1 slot assignment (relation-major) ----
        o1 = np.argsort(gk, kind="stable")
        g1, s1, d1 = gk[o1], sk[o1], dl[o1]
        gstart = np.zeros(NG + 1, np.int64)
        np.cumsum(np.bincount(g1, minlength=NG), out=gstart[1:])
        lpos = np.arange(len(g1)) - gstart[g1]          # pos within group
        c1 = g1 * CH_G + lpos // P
        p1 = lpos % P
        src_packed = np.zeros((P, CH1), np.int32)
        src_packed[p1, c1] = s1
        rowid = (p1 * CH1 + c1).astype(np.int64)         # msgs DRAM row of edge

        # ---- phase-2 slot assignment (destination-major) ----
        o2 = np.argsort(d1, kind="stable")
        d2, r2 = d1[o2], rowid[o2]
        blk = d2 // P
        bstart = np.zeros(n_blocks + 1, np.int64)
        np.cumsum(np.bincount(blk, minlength=n_blocks), out=bstart[1:])
        bpos = np.arange(len(d2)) - bstart[blk]
        c2 = blk * CH_B + bpos // P
        p2 = bpos % P
        perm_packed = np.zeros((P, CH2), np.int32)
        dstoff_packed = np.full((P, CH2), 255, np.float32)
        perm_packed[p2, c2] = r2
        dstoff_packed[p2, c2] = d2 % P

        shards.append({
            "src": src_packed,
            "perm": perm_packed,
            "dstoff": dstoff_packed,
            "xself": np.ascontiguousarray(x[k * Npc:(k + 1) * Npc]),
        })

    xdt = ml_dtypes.bfloat16 if X_BF16 else np.float32
    wstk = np.ascontiguousarray(
        np.transpose(W[2:2 + NG], (1, 0, 2)).reshape(D, NG * D).astype(xdt))
    w0 = np.ascontiguousarray(W[0].astype(xdt))
    bstk = np.ascontiguousarray(
        b[2:2 + NG].reshape(1, NG * D).astype(ml_dtypes.bfloat16))
    b0 = np.ascontiguousarray(b[0].reshape(1, D).astype(ml_dtypes.bfloat16))

    shared = {"x": np.ascontiguousarray(x.astype(xdt)),
              "wstk": wstk, "w0": w0, "bstk": bstk, "b0": b0}
    for sh in shards:
        sh["xself"] = sh["xself"].astype(xdt)
    meta = dict(N=N, D=D, NG=NG, Npc=Npc, n_blocks=n_blocks,
                CH_G=CH_G, CH_B=CH_B, CH1=CH1, CH2=CH2)
    return shared, shards, meta


# ----------------------------------------------------------------------------
# Device program
# ----------------------------------------------------------------------------

def _build_program(meta, bench_reps=0):
    import contextlib
    import concourse.bass as bass
    import concourse.bacc as bacc
    import concourse.mybir as mybir
    import concourse.tile as tile
    from concourse.masks import make_identity

    D = meta["D"]
    NG = meta["NG"]
    N = meta["N"]
    Npc = meta["Npc"]
    n_blocks = meta["n_blocks"]
    CH_G, CH_B, CH1, CH2 = meta["CH_G"], meta["CH_B"], meta["CH1"], meta["CH2"]

    f32 = mybir.dt.float32
    bf16 = mybir.dt.bfloat16
    i32 = mybir.dt.int32
    xdt = bf16 if X_BF16 else f32

    # phase-1 pieces: chunks per gather piece
    PC1 = 32
    while CH1 % PC1:
        PC1 //= 2
    NP1 = CH1 // PC1
    # phase-2 pieces: whole number of blocks per piece
    BPP = 8
    while n_blocks % BPP:
        BPP //= 2
    PC2 = BPP * CH_B
    NP2 = n_blocks // BPP

    nc = bacc.Bacc(None, target_bir_lowering=False)

    x_d = nc.dram_tensor("x", [N, D], xdt, kind="ExternalInput")
    xself_d = nc.dram_tensor("xself", [Npc, D], xdt, kind="ExternalInput")
    wstk_d = nc.dram_tensor("wstk", [D, NG * D], xdt, kind="ExternalInput")
    w0_d = nc.dram_tensor("w0", [D, D], xdt, kind="ExternalInput")
    bstk_d = nc.dram_tensor("bstk", [1, NG * D], bf16, kind="ExternalInput")
    b0_d = nc.dram_tensor("b0", [1, D], bf16, kind="ExternalInput")
    src_d = nc.dram_tensor("src", [P, CH1], i32, kind="ExternalInput")
    perm_d = nc.dram_tensor("perm", [P, CH2], i32, kind="ExternalInput")
    dstoff_d = nc.dram_tensor("dstoff", [P, CH2], f32, kind="ExternalInput")
    msgs_d = nc.dram_tensor("msgs", [P * CH1, D], bf16, kind="Internal")
    out_d = nc.dram_tensor("out", [Npc, D], f32, kind="ExternalOutput")

    msgs_v = msgs_d[:, :].rearrange("(p c) d -> p c d", p=P)
    xself_v = xself_d[:, :].rearrange("(b p) d -> p b d", p=P)
    out_v = out_d[:, :].rearrange("(b p) d -> p b d", p=P)

    with tile.TileContext(nc) as tc:
        with tc.tile_pool(name="const", bufs=1) as cpool:
            ident = cpool.tile([P, P], xdt)
            make_identity(nc, ident[:])
            iota_t = cpool.tile([P, P], f32)
            nc.gpsimd.iota(iota_t[:], pattern=[[1, P]], base=0,
                           channel_multiplier=0,
                           allow_small_or_imprecise_dtypes=True)
            ones_t = cpool.tile([1, P], bf16)
            nc.vector.memset(ones_t[:], 1.0)
            wsb = cpool.tile([P, NG * D], xdt)
            nc.sync.dma_start(out=wsb[:], in_=wstk_d[:, :])
            w0sb = cpool.tile([P, D], xdt)
            nc.sync.dma_start(out=w0sb[:], in_=w0_d[:, :])
            bsb = cpool.tile([1, NG * D], bf16)
            nc.sync.dma_start(out=bsb[:], in_=bstk_d[:, :])
            b0sb = cpool.tile([1, D], bf16)
            nc.sync.dma_start(out=b0sb[:], in_=b0_d[:, :])
            srcsb = cpool.tile([P, CH1], i32)
            nc.sync.dma_start(out=srcsb[:], in_=src_d[:, :])
            permsb = cpool.tile([P, CH2], i32)
            nc.sync.dma_start(out=permsb[:], in_=perm_d[:, :])
            dstsb = cpool.tile([P, CH2], f32)
            nc.sync.dma_start(out=dstsb[:], in_=dstoff_d[:, :])

            loop_cm = (tc.For_i(0, bench_reps, 1) if bench_reps
                       else contextlib.nullcontext())
            with loop_cm:
                _build_body(nc, tc, meta, locals())

    nc.finalize()
    return nc


def _build_body(nc, tc, meta, env):
    import concourse.bass as bass
    import concourse.mybir as mybir

    D = meta["D"]
    CH_G, CH_B = meta["CH_G"], meta["CH_B"]
    PC1, NP1 = env["PC1"], env["NP1"]
    PC2, NP2, BPP = env["PC2"], env["NP2"], env["BPP"]
    f32, bf16 = mybir.dt.float32, mybir.dt.bfloat16
    xdt = env["xdt"]
    x_d, msgs_d = env["x_d"], env["msgs_d"]
    msgs_v, xself_v, out_v = env["msgs_v"], env["xself_v"], env["out_v"]
    ident, iota_t, ones_t = env["ident"], env["iota_t"], env["ones_t"]
    wsb, w0sb, bsb, b0sb = env["wsb"], env["w0sb"], env["bsb"], env["b0sb"]
    srcsb, permsb, dstsb = env["srcsb"], env["permsb"], env["dstsb"]

    if True:
        if True:
            # ---------------- phase 1: messages ----------------
            with tc.tile_pool(name="p1big", bufs=3) as p1big, \
                 tc.tile_pool(name="p1small", bufs=10) as p1small, \
                 tc.tile_pool(name="ps_t", bufs=3, space="PSUM") as ps_t, \
                 tc.tile_pool(name="ps_m", bufs=3, space="PSUM") as ps_m:
                for piece in range(NP1):
                    mp = p1big.tile([P, PC1 * D], bf16, tag="mp")
                    for cl in range(PC1):
                        c = piece * PC1 + cl
                        g = c // CH_G
                        xg = p1small.tile([P, D], xdt, tag="xg")
                        nc.gpsimd.indirect_dma_start(
                            out=xg[:],
                            out_offset=None,
                            in_=x_d[:, :],
                            in_offset=bass.IndirectOffsetOnAxis(
                                ap=srcsb[:, c:c + 1], axis=0),
                        )
                        xgT_ps = ps_t.tile([P, P], xdt, tag="t")
                        nc.tensor.transpose(xgT_ps[:], xg[:], ident[:])
                        xgT = p1small.tile([P, P], xdt, tag="xgT")
                        nc.any.tensor_copy(xgT[:], xgT_ps[:])
                        mm_ps = ps_m.tile([P, P], f32, tag="m")
                        nc.tensor.matmul(
                            mm_ps[:], lhsT=xgT[:],
                            rhs=wsb[:, g * D:(g + 1) * D],
                            start=True, stop=False)
                        nc.tensor.matmul(
                            mm_ps[:], lhsT=ones_t[:],
                            rhs=bsb[:, g * D:(g + 1) * D],
                            start=False, stop=True)
                        nc.any.tensor_copy(mp[:, cl * D:(cl + 1) * D], mm_ps[:])
                    nc.sync.dma_start(
                        out=msgs_v[:, piece * PC1:(piece + 1) * PC1, :],
                        in_=mp[:])

            # ---------------- phase 2: aggregate ----------------
            with tc.tile_pool(name="p2big", bufs=3) as p2big, \
                 tc.tile_pool(name="p2small", bufs=10) as p2small, \
                 tc.tile_pool(name="ps_t2", bufs=2, space="PSUM") as ps_t2, \
                 tc.tile_pool(name="ps_o", bufs=3, space="PSUM") as ps_o:
                for piece in range(NP2):
                    xs = p2big.tile([P, BPP * D], xdt, tag="xs")
                    nc.sync.dma_start(
                        out=xs[:],
                        in_=xself_v[:, piece * BPP:(piece + 1) * BPP, :])
                    ob = p2big.tile([P, BPP * D], f32, tag="ob")
                    for bl in range(BPP):
                        blk_ps = ps_o.tile([P, P], f32, tag="o")
                        # self transform
                        xsT_ps = ps_t2.tile([P, P], xdt, tag="t2")
                        nc.tensor.transpose(
                            xsT_ps[:], xs[:, bl * D:(bl + 1) * D], ident[:])
                        xsT = p2small.tile([P, P], xdt, tag="xsT")
                        nc.any.tensor_copy(xsT[:], xsT_ps[:])
                        nc.tensor.matmul(blk_ps[:], lhsT=xsT[:], rhs=w0sb[:],
                                         start=True, stop=False)
                        nc.tensor.matmul(blk_ps[:], lhsT=ones_t[:],
                                         rhs=b0sb[:], start=False, stop=False)
                        for j in range(CH_B):
                            c2 = piece * PC2 + bl * CH_B + j
                            mg = p2small.tile([P, D], bf16, tag="mg")
                            nc.gpsimd.indirect_dma_start(
                                out=mg[:],
                                out_offset=None,
                                in_=msgs_d[:, :],
                                in_offset=bass.IndirectOffsetOnAxis(
                                    ap=permsb[:, c2:c2 + 1], axis=0),
                            )
                            a_t = p2small.tile([P, P], bf16, tag="a")
                            nc.vector.tensor_scalar(
                                out=a_t[:], in0=iota_t[:],
                                scalar1=dstsb[:, c2:c2 + 1],
                                scalar2=None,
                                op0=mybir.AluOpType.is_equal)
                            nc.tensor.matmul(
                                blk_ps[:], lhsT=a_t[:],
                                rhs=mg[:],
                                start=False, stop=(j == CH_B - 1))
                        nc.scalar.activation(
                            ob[:, bl * D:(bl + 1) * D], blk_ps[:],
                            mybir.ActivationFunctionType.Relu)
                    nc.sync.dma_start(
                        out=out_v[:, piece * BPP:(piece + 1) * BPP, :],
                        in_=ob[:])


# ----------------------------------------------------------------------------
# Entry point
# ----------------------------------------------------------------------------

def _run_timed(nc, in_maps, n_cores=8, iters=1):
    """Execute a finalized Bass program via PJRT on n_cores devices.

    Returns (per-core results, best wall seconds of a repeat execution).
    """
    import time
    import jax
    from jax.sharding import Mesh, PartitionSpec, NamedSharding
    from jax.experimental.shard_map import shard_map
    import concourse.mybir as mybir
    from concourse import bass2jax

    bass2jax.install_neuronx_cc_hook()
    partition_name = (nc.partition_id_tensor.name
                      if nc.partition_id_tensor else None)

    in_names, out_names, out_avals, zero_outs = [], [], [], []
    for alloc in nc.m.functions[0].allocations:
        if not isinstance(alloc, mybir.MemoryLocationSet):
            continue
        name = alloc.memorylocations[0].name
        if alloc.kind == "ExternalInput":
            if name != partition_name:
                in_names.append(name)
        elif alloc.kind == "ExternalOutput":
            out_names.append(name)
            shape = tuple(alloc.tensor_shape)
            dtype = mybir.dt.np(alloc.dtype)
            out_avals.append(jax.core.ShapedArray(shape, dtype))
            zero_outs.append(np.zeros(shape, dtype))
    n_params = len(in_names)
    n_outs = len(out_avals)
    all_names = in_names + out_names
    if partition_name is not None:
        all_names = all_names + [partition_name]

    def _body(*args):
        operands = list(args)
        if partition_name is not None:
            operands.append(bass2jax.partition_id_tensor())
        outs = bass2jax._bass_exec_p.bind(
            *operands,
            out_avals=tuple(out_avals),
            in_names=tuple(all_names),
            out_names=tuple(out_names),
            lowering_input_output_aliases=(),
            sim_require_finite=True,
            sim_require_nnan=True,
            nc=nc,
        )
        return tuple(outs)

    devices = jax.devices()[:n_cores]
    mesh = Mesh(np.asarray(devices), ("core",))
    spec = PartitionSpec("core")
    sharded = jax.jit(
        shard_map(_body, mesh=mesh, in_specs=(spec,) * (n_params + n_outs),
                  out_specs=(spec,) * n_outs, check_rep=False),
        keep_unused=True)

    sh = NamedSharding(mesh, spec)
    concat_in = [
        jax.device_put(
            np.concatenate([np.asarray(in_maps[c][n]) for c in range(n_cores)],
                           axis=0), sh)
        for n in in_names
    ]
    concat_zeros = [
        jax.device_put(np.zeros((n_cores * z.shape[0], *z.shape[1:]), z.dtype),
                       sh)
        for z in zero_outs
    ]

    out_arrs = jax.block_until_ready(sharded(*concat_in, *concat_zeros))
    best = float("inf")
    for _ in range(iters):
        t0 = time.perf_counter()
        out_arrs = jax.block_until_ready(sharded(*concat_in, *concat_zeros))
        best = min(best, time.perf_counter() - t0)

    results = [
        {name: np.asarray(out_arrs[i]).reshape(n_cores, *out_avals[i].shape)[c]
         for i, name in enumerate(out_names)}
        for c in range(n_cores)
    ]
    return results, best


last_exec_seconds = None  # wall seconds of the most recent run (for test.py)


def kernel(x, W, b, dep_idx, gov_idx, rel_idx):
    n_cores = 8
    shared, shards, meta = _prepare_host(
        np.asarray(x), np.asarray(W), np.asarray(b),
        dep_idx, gov_idx, rel_idx, n_cores)

    nc = _build_program(meta)

    in_maps = [dict(shared, **sh) for sh in shards]
    results, best = _run_timed(nc, in_maps, n_cores=n_cores, iters=3)
    global last_exec_seconds
    last_exec_seconds = best
    out = np.concatenate([r["out"] for r in results], axis=0)
    return out.astype(np.float32)
